# revision 33
# baseline (speedup 1.0000x reference)
"""Trainium2 Bass kernel v3.2: occupancy-class packed sparse conv.

Math: out[j, d] = sum over points i with out_idx[i]==j of  x[i, :] @ W[k_idx[i], :, d]

Class layout per core (columns sorted by class, then pattern; voxels
dealt round-robin across the 8 cores so all cores share one
compile-time program; padded columns are zero and never gathered):
  m=1: 32-row slab, 8 runs, stationary = W[k] slices (16KB wq)
  m=2: 64-row slab, C(8,2)=28-pattern dict [64,32] (112KB)
  m=3..4: 128-row slab, mask padded to 4 slots, C(8,4)=70-pattern dict
  m>=5: dense 256-row slab (two 128-row DMAs), 2 matmuls vs W-halves

The earlier fully-exact variant (per-m classes + pattern-compressed
m>=5) cut HBM bytes to 9.5MB/core but needed 429 matmuls; measured
~65ns/matmul overhead made Tensor the critical path (80us). This
version trades ~1.1MB of padding back for ~170 matmuls.
"""
import sys

if "/opt/trn_rl_repo" not in sys.path:
    sys.path.insert(0, "/opt/trn_rl_repo")

import numpy as np

N_CORES = 8
PSUM_W = 512           # psum tile width (voxel columns); group = 4 tiles = 2048
GROUP_W = 4 * PSUM_W

POP = np.array([bin(i).count("1") for i in range(256)], dtype=np.int64)
PATS = {m: [b for b in range(256) if POP[b] == m] for m in (1, 2, 3, 4)}
PAT_ID = {m: np.full(256, -1, dtype=np.int64) for m in (1, 2, 3, 4)}
for _m in (1, 2, 3, 4):
    for _i, _b in enumerate(PATS[_m]):
        PAT_ID[_m][_b] = _i
N_PAT = {m: len(PATS[m]) for m in (1, 2, 3, 4)}   # 8, 28, 56, 70

# whole-slab DMAs: per-partition descriptor = full slab row (7-19KB);
# per-queue throughput rises steeply with descriptor size (2-5KB -> 50-90
# GB/s, 8-18KB -> 100-140GB/s per queue, 3 queues concurrent)
CHUNK = {1: 16384, 21: 16384, 22: 16384, 3: 16384, 4: 16384, 5: 16384}
ROWS = {1: 32, 21: 64, 22: 64, 3: 96, 4: 128, 5: 128}
# SBUF partition band per class (per-partition DMA write bandwidth ~2.6B/ns
# is the stream-phase limit; class 2 is split into two half-column bands
# and class 1 moved high so no partition gets two narrow slabs)
POFF = {1: 96, 21: 0, 22: 64, 3: 0, 4: 0, 5: 0}

_prog_cache = {}


def _build_program(desc):
    import concourse.tile as tile
    from concourse import bacc, mybir

    bf16 = mybir.dt.bfloat16
    f32 = mybir.dt.float32
    L = desc["L"]            # class id (1,2,4,5) -> per-core columns
    Ntot = desc["Ntot"]
    ngroups = desc["ngroups"]
    segs = desc["segs"]

    nc = bacc.Bacc("TRN2", target_bir_lowering=False, debug=False)
    slab_d = {}
    for c in (1, 21, 22, 3, 4, 5):
        slab_d[c] = nc.dram_tensor(f"slab{c}", [ROWS[c], max(L[c], 1)], bf16,
                                   kind="ExternalInput")
    bhi_d = nc.dram_tensor("slabhi", [128, max(L[5], 1)], bf16, kind="ExternalInput")
    wq_d = nc.dram_tensor("wq", [32, 8 * 32], bf16, kind="ExternalInput")
    d2_d = nc.dram_tensor("dict2", [64, N_PAT[2] * 32], bf16, kind="ExternalInput")
    d3_d = nc.dram_tensor("dict3", [96, N_PAT[3] * 32], bf16, kind="ExternalInput")
    d4_d = nc.dram_tensor("dict4", [128, N_PAT[4] * 32], bf16, kind="ExternalInput")
    wf_d = nc.dram_tensor("wflat", [256, 32], bf16, kind="ExternalInput")
    nblocks = -(-ngroups // 2)
    out_d = nc.dram_tensor("out_st", [nblocks, 128, 2 * PSUM_W], bf16,
                           kind="ExternalOutput")

    with tile.TileContext(nc) as tc:
        with (
            tc.tile_pool(name="w", bufs=1) as wpool,
            tc.tile_pool(name="s1", bufs=1) as s1pool,
            tc.tile_pool(name="s21", bufs=1) as s21pool,
            tc.tile_pool(name="s22", bufs=1) as s22pool,
            tc.tile_pool(name="s3", bufs=1) as s3pool,
            tc.tile_pool(name="s4", bufs=1) as s4pool,
            tc.tile_pool(name="s5", bufs=1) as s5pool,
            tc.tile_pool(name="st", bufs=6) as stpool,
            tc.tile_pool(name="ps", bufs=6, space="PSUM") as pspool,
        ):
            wq_t = wpool.tile([128, 8 * 32], bf16, tag="wq")
            d2_t = wpool.tile([128, N_PAT[2] * 32], bf16, tag="d2")
            d3_t = wpool.tile([96, N_PAT[3] * 32], bf16, tag="d3")
            d4_t = wpool.tile([128, N_PAT[4] * 32], bf16, tag="d4")
            wf0 = wpool.tile([128, 32], bf16, tag="wf0")
            wf1 = wpool.tile([128, 32], bf16, tag="wf1")
            dict_t = {1: wq_t, 21: d2_t, 22: d2_t, 3: d3_t, 4: d4_t}

            # sync+scalar carry the slab inputs (greedy byte-balance);
            # gpsimd (SWDGE) carries outputs + tiny early inputs so staged
            # outputs never queue behind bulk inputs
            queues = [nc.sync, nc.scalar, nc.gpsimd]
            qbytes = [0, 0, 0]

            def dma(dst, src, nbytes, q=None):
                i = qbytes.index(min(qbytes)) if q is None else q
                qbytes[i] += nbytes
                queues[i].dma_start(dst, src)

            pools = {1: s1pool, 21: s21pool, 22: s22pool, 3: s3pool,
                     4: s4pool, 5: s5pool}
            chunks = {}
            # all weight dicts FIRST (small; a dict stuck behind a slab
            # stalls that class's compute -> staged-output head-of-line)
            dma(wq_t[96:128, :], wq_d.ap()[:, :], 32 * 256 * 2, q=2)
            dma(d2_t[0:64, :], d2_d.ap()[:, :], 64 * N_PAT[2] * 32 * 2)
            dma(d2_t[64:128, :], d2_d.ap()[:, :], 64 * N_PAT[2] * 32 * 2)
            dma(d3_t[:], d3_d.ap()[:, :], 96 * N_PAT[3] * 32 * 2)
            dma(d4_t[:], d4_d.ap()[:, :], 128 * N_PAT[4] * 32 * 2)
            dma(wf0[:], wf_d.ap()[0:128, :], 128 * 64, q=2)
            dma(wf1[:], wf_d.ap()[128:256, :], 128 * 64, q=2)

            # DMA plan: each input chunk tagged with the group where its
            # columns are first consumed. Chunks for early groups are issued
            # upfront; the rest just-in-time (PREFETCH groups of lookahead)
            # inside the compute loop, so output DMAs are not enqueued
            # behind the entire input backlog (the DMA subsystem services
            # descriptors roughly in enqueue order).
            PREFETCH = 6
            plan = []

            def emit_chunk(c, i):
                lo = i * CHUNK[c]
                hi = min(max(L[c], 1), lo + CHUNK[c])
                po = POFF[c]
                t = pools[c].tile([po + ROWS[c], hi - lo], bf16,
                                  tag=f"c{c}_{i}")
                dma(t[po:po + ROWS[c], :], slab_d[c].ap()[:, lo:hi],
                    ROWS[c] * (hi - lo) * 2)
                chunks[(c, i)] = t
                if c == 5:
                    th = pools[5].tile([128, hi - lo], bf16, tag=f"c6_{i}")
                    dma(th[:], bhi_d.ap()[:, lo:hi], 128 * (hi - lo) * 2)
                    chunks[(6, i)] = th

            for c in desc["order"]:
                Lc = max(L[c], 1)
                n = max(1, -(-Lc // CHUNK[c]))
                for i in range(n):
                    fcg = (desc["cstart"][c] + i * CHUNK[c]) // GROUP_W
                    plan.append((fcg, c, i))
            plan.sort(key=lambda e: e[0])
            pi = 0
            while pi < len(plan) and plan[pi][0] <= PREFETCH:
                emit_chunk(plan[pi][1], plan[pi][2])
                pi += 1

            copy_i = 0
            staging = None
            for g in range(ngroups):
                while pi < len(plan) and plan[pi][0] - PREFETCH <= g:
                    emit_chunk(plan[pi][1], plan[pi][2])
                    pi += 1
                if g % 2 == 0:
                    staging = stpool.tile([128, 2 * PSUM_W], bf16)
                ps = pspool.tile([128, PSUM_W], f32)
                for a in range(4):
                    T = 4 * g + a
                    if PSUM_W * T >= Ntot:
                        continue
                    for s in segs[T]:
                        if s[0] == 0:
                            _, c, o, w, pid, ci = s
                            off = (PSUM_W * T + o - desc["cstart"][c]
                                   - ci * CHUNK[c])
                            po = POFF[c]
                            nc.tensor.matmul(
                                ps[32 * a:32 * a + 32, o:o + w],
                                dict_t[c][po:po + ROWS[c],
                                          32 * pid:32 * pid + 32],
                                chunks[(c, ci)][po:po + ROWS[c],
                                                off:off + w],
                                start=True, stop=True,
                                tile_position=(po, 32 * a),
                            )
                        else:
                            _, o, w, ci = s
                            off = (PSUM_W * T + o - desc["cstart"][5]
                                   - ci * CHUNK[5])
                            nc.tensor.matmul(
                                ps[32 * a:32 * a + 32, o:o + w],
                                wf0, chunks[(5, ci)][:, off:off + w],
                                start=True, stop=False,
                                tile_position=(0, 32 * a),
                            )
                            nc.tensor.matmul(
                                ps[32 * a:32 * a + 32, o:o + w],
                                wf1, chunks[(6, ci)][:, off:off + w],
                                start=False, stop=True,
                                tile_position=(0, 32 * a),
                            )
                dst = staging[:, PSUM_W * (g % 2):PSUM_W * (g % 2 + 1)]
                if copy_i % 2 == 0:
                    nc.vector.tensor_copy(dst, ps[:])
                else:
                    nc.scalar.copy(dst, ps[:])
                copy_i += 1
                if g % 2 == 1 or g == ngroups - 1:
                    dma(out_d.ap()[g // 2], staging[:, :], 128 * 1024 * 2)

    nc.compile()
    return nc


def _get_program(desc):
    key = (tuple(sorted(desc["L"].items())), desc["Ntot"], desc["ngroups"],
           tuple(desc["order"]), tuple(sorted(desc["cstart"].items())),
           tuple(tuple(map(tuple, s)) for s in desc["segs"]))
    if key not in _prog_cache:
        _prog_cache[key] = _build_program(desc)
    return _prog_cache[key]


def _pack(x, W, k_idx, out_idx, num_out):
    """Host-side packing. Returns (in_maps, desc, vox_core, vox_col)."""
    import ml_dtypes
    bf = ml_dtypes.bfloat16
    n = x.shape[0]

    masks = np.zeros(num_out, np.uint8)
    np.bitwise_or.at(masks, out_idx, (np.uint8(1) << k_idx.astype(np.uint8)))
    masks64 = masks.astype(np.int64)
    m = POP[masks]

    cls = np.where(m <= 4, m, 5)

    vox_core = np.empty(num_out, np.int64)
    vox_col = np.empty(num_out, np.int64)

    CLS_ORDER = [21, 22, 3, 4, 5, 1]
    L = {}
    class_start = {}
    col_cursor = 0
    run_lists = {}
    for c in CLS_ORDER:
        if c == 5:
            idsB = np.where(cls == 5)[0]
            rB = np.arange(len(idsB))
            vox_core[idsB] = rB % N_CORES
            vox_col[idsB] = col_cursor + rB // N_CORES
            L[5] = int(-(-len(idsB) // N_CORES)) if len(idsB) else 0
            class_start[5] = col_cursor
            col_cursor += L[5]
            continue
        cm = 2 if c in (21, 22) else c
        ids = np.where(cls == cm)[0]
        pid = PAT_ID[cm][masks64[ids]]
        if cm == 2:
            # split class 2 voxels into two balanced halves per pattern
            o1 = np.argsort(pid, kind="stable")
            st = np.concatenate([[0], np.cumsum(np.bincount(
                pid[o1], minlength=N_PAT[2]))[:-1]])
            r1 = np.arange(len(ids)) - st[pid[o1]]
            half = (r1 // N_CORES) % 2
            keep = o1[half == (0 if c == 21 else 1)]
            ids, pid = ids[keep], pid[keep]
        order = np.argsort(pid, kind="stable")
        sids, spid = ids[order], pid[order]
        cnts = np.bincount(spid, minlength=N_PAT[cm])
        Lp = -(-cnts // N_CORES)
        offs = np.concatenate([[0], np.cumsum(Lp)[:-1]])
        starts = np.concatenate([[0], np.cumsum(cnts)[:-1]])
        r = np.arange(len(sids)) - starts[spid]
        vox_core[sids] = r % N_CORES
        vox_col[sids] = col_cursor + offs[spid] + r // N_CORES
        L[c] = int(Lp.sum())
        class_start[c] = col_cursor
        col_cursor += L[c]
        run_lists[c] = [(p, int(Lp[p])) for p in range(N_PAT[cm]) if Lp[p] > 0]
    Ntot = col_cursor
    ngroups = max(1, -(-Ntot // GROUP_W))

    nT = ngroups * 4
    segs = [[] for _ in range(nT)]

    def add_runs(c, runs):
        base = class_start[c]
        off = 0
        for key, run_len in runs:
            lo = base + off
            run_hi = lo + run_len
            off += run_len
            while lo < run_hi:
                T = lo // PSUM_W
                nb_t = (T + 1) * PSUM_W
                nb_c = base + ((lo - base) // CHUNK[c] + 1) * CHUNK[c]
                hi = min(run_hi, nb_t, nb_c)
                o = lo - PSUM_W * T
                ci = (lo - base) // CHUNK[c]
                if c != 5:
                    segs[T].append((0, c, o, hi - lo, key, ci))
                else:
                    segs[T].append((1, o, hi - lo, ci))
                lo = hi

    for c in CLS_ORDER:
        if c == 5:
            if L[5]:
                add_runs(5, [(None, L[5])])
        else:
            add_runs(c, run_lists[c])

    # --- fill slabs ---
    pairs = out_idx.astype(np.int64) * 8 + k_idx
    unique_pairs = np.unique(pairs).size == n

    v = out_idx.astype(np.int64)
    k64 = k_idx.astype(np.int64)
    cv = cls[v]
    mk64 = masks64[v]
    slabs = {}
    colv = vox_col[v]
    for c in (1, 21, 22, 3, 4):
        cm = 2 if c in (21, 22) else c
        sel = cv == cm
        if cm == 2:
            lo_, hi_ = class_start[c], class_start[c] + L[c]
            sel = sel & (colv >= lo_) & (colv < hi_)
        xa, va, ka = x[sel], v[sel], k64[sel]
        rank = POP[mk64[sel] & ((np.int64(1) << ka) - 1)]
        sl = np.zeros((N_CORES, max(L[c], 1), ROWS[c] // 32, 32), np.float32)
        idx = (vox_core[va], vox_col[va] - class_start[c], rank)
        if unique_pairs:
            sl[idx] = xa
        else:
            np.add.at(sl, idx, xa)
        slabs[c] = sl
    selB = cv == 5
    xb, vb, kb = x[selB], v[selB], k64[selB]
    sb = np.zeros((N_CORES, max(L[5], 1), 8, 32), np.float32)
    ib = (vox_core[vb], vox_col[vb] - class_start[5], kb)
    if unique_pairs:
        sb[ib] = xb
    else:
        np.add.at(sb, ib, xb)

    def build_dict(mm):
        P = N_PAT[mm]
        wd = np.zeros((mm, 32, P, 32), np.float32)
        for p, byte in enumerate(PATS[mm]):
            ks = [k for k in range(8) if (byte >> k) & 1]
            for rr, k in enumerate(ks):
                wd[rr, :, p, :] = W[k]
        return np.ascontiguousarray(wd.reshape(mm * 32, P * 32)).astype(bf)

    wq = np.ascontiguousarray(W.transpose(1, 0, 2).reshape(32, 8 * 32)).astype(bf)
    d2, d3, d4 = build_dict(2), build_dict(3), build_dict(4)
    wflat = W.reshape(256, 32).astype(bf)

    in_maps = []
    for c in range(N_CORES):
        im = {"wq": wq, "dict2": d2, "dict3": d3, "dict4": d4, "wflat": wflat}
        for cl in (1, 21, 22, 3, 4):
            im[f"slab{cl}"] = np.ascontiguousarray(
                slabs[cl][c].reshape(max(L[cl], 1), ROWS[cl]).T).astype(bf)
        sbc = sb[c].reshape(max(L[5], 1), 256).T     # [256, L5]
        im["slab5"] = np.ascontiguousarray(sbc[0:128]).astype(bf)
        im["slabhi"] = np.ascontiguousarray(sbc[128:256]).astype(bf)
        in_maps.append(im)
    desc = {"L": L, "Ntot": Ntot, "ngroups": ngroups, "segs": segs,
            "order": CLS_ORDER, "cstart": class_start}
    return in_maps, desc, vox_core, vox_col


def _decode(results, desc, vox_core, vox_col):
    ngroups = desc["ngroups"]
    nblocks = -(-ngroups // 2)
    NT = nblocks * 2 * GROUP_W
    outs = []
    for rres in results:
        st = np.asarray(rres["out_st"], dtype=np.float32)   # [b, 128, 1024]
        arr = st.reshape(nblocks, 4, 32, 2, PSUM_W)         # [b, a, d, gg, t]
        outT = arr.transpose(2, 0, 3, 1, 4).reshape(32, NT)  # col=4096b+2048gg+512a+t
        outs.append(outT)
    full = np.stack(outs)
    return np.ascontiguousarray(full[vox_core, :, vox_col])


def run(x, W, k_idx, out_idx, num_out, trace=False, dt_name=None):
    from concourse.bass_utils import run_bass_kernel_spmd

    x = np.asarray(x, dtype=np.float32)
    W = np.asarray(W, dtype=np.float32)
    k_idx = np.asarray(k_idx, dtype=np.int32)
    out_idx = np.asarray(out_idx, dtype=np.int32)
    num_out = int(num_out)

    in_maps, desc, vox_core, vox_col = _pack(x, W, k_idx, out_idx, num_out)
    nc = _get_program(desc)
    res = run_bass_kernel_spmd(nc, in_maps, list(range(N_CORES)), trace=trace)
    out = _decode(res.results, desc, vox_core, vox_col)
    return out, res


def kernel(x, W, k_idx, out_idx, num_out):
    out, _ = run(x, W, k_idx, out_idx, num_out, trace=False)
    return out


# revision 35
# speedup vs baseline: 1.0832x; 1.0832x over previous
"""Trainium2 Bass kernel v2: pattern-compressed sparse-conv gather-GEMM-scatter.

Math: out[j, d] = sum over points i with out_idx[i]==j of  x[i, :] @ W[k_idx[i], :, d]

v1 ("dense k-slot expansion") sent 8 slots x 32ch = 256 values per output
voxel; at ~38% slot occupancy that wastes ~62% of HBM read traffic on zeros,
and the kernel sits at the per-core DMA roofline.

v2 ("pattern compression"): each output voxel j has an occupied-slot mask
(which of the 8 kernel offsets have a point). ~85% of voxels have <= 4
occupied slots. Those are packed as 4 slot-blocks (128 values) under one of
C(8,4)=70 canonical patterns; the matmul stationary for pattern p is the
[128, 32] stack of W[k] rows for p's slots (host-built dictionary). Voxels
with >= 5 slots use the full dense-256 layout (two accumulated matmuls).

Voxels are sorted by (class, pattern) and dealt round-robin across the 8
cores, so all cores share an identical compile-time program structure (runs
differ by <= 1 column, zero-padded). Output voxel order is the sorted order;
the host inverse-permutes after gather (free).

Per-core traffic: ~7.0 MB (A slab) + ~2.4 MB (B slab) + 0.57 MB (pattern
weight dict) in, ~2.5 MB out (bf16) -- vs 21 MB for v1. Tensor: ~37k PE
cycles vs 65.5k.
"""
import sys

if "/opt/trn_rl_repo" not in sys.path:
    sys.path.insert(0, "/opt/trn_rl_repo")

import numpy as np

N_CORES = 8
CHUNK = 4096        # slabA DMA chunk (columns)
PSUM_W = 512        # psum tile width (voxel columns)
STAGE_W = 2048      # staging tile [128, 2048] = 8192 voxel columns

POP = np.array([bin(i).count("1") for i in range(256)], dtype=np.int64)
PAT4 = [b for b in range(256) if POP[b] == 4]           # 70 patterns
PAT_ID = np.full(256, -1, dtype=np.int64)
for _i, _b in enumerate(PAT4):
    PAT_ID[_b] = _i

_prog_cache = {}


def _build_program(desc):
    import concourse.tile as tile
    from concourse import bacc, mybir

    bf16 = mybir.dt.bfloat16
    f32 = mybir.dt.float32
    NAc, LB, Ntot, nb = desc["NAc"], desc["LB"], desc["Ntot"], desc["nb"]
    segs = desc["segs"]
    nchunks = max(1, -(-NAc // CHUNK))

    nc = bacc.Bacc("TRN2", target_bir_lowering=False, debug=False)
    slabA_d = nc.dram_tensor("slabA", [128, max(NAc, 1)], bf16, kind="ExternalInput")
    slabB_d = nc.dram_tensor("slabB", [256, max(LB, 1)], bf16, kind="ExternalInput")
    wdict_d = nc.dram_tensor("wdict", [128, 70 * 32], bf16, kind="ExternalInput")
    wflat_d = nc.dram_tensor("wflat", [256, 32], bf16, kind="ExternalInput")
    out_d = nc.dram_tensor("out_st", [nb, 128, STAGE_W], bf16, kind="ExternalOutput")

    with tile.TileContext(nc) as tc:
        with (
            tc.tile_pool(name="w", bufs=1) as wpool,
            tc.tile_pool(name="a", bufs=1) as apool,
            tc.tile_pool(name="bb", bufs=1) as bpool,
            tc.tile_pool(name="st", bufs=2) as stpool,
            tc.tile_pool(name="ps", bufs=4, space="PSUM") as pspool,
        ):
            # one merged weight tile: 70 pattern stationaries + 2 wflat halves
            wdict_t = wpool.tile([128, 72 * 32], bf16, tag="wdict")
            wf0 = wdict_t[:, 70 * 32:71 * 32]
            wf1 = wdict_t[:, 71 * 32:72 * 32]
            nc.sync.dma_start(wdict_t[:, 0:70 * 32], wdict_d.ap()[:, :])
            nc.sync.dma_start(wf0, wflat_d.ap()[0:128, :])
            nc.sync.dma_start(wf1, wflat_d.ap()[128:256, :])

            # gpsimd queue: earliest-consumed chunks only, then output DMAs;
            # remaining input byte-balanced on sync/scalar
            qload = {"sync": 0.6e6, "scalar": 0.0}
            qeng = {"sync": nc.sync, "scalar": nc.scalar}
            chA = []
            for i in range(nchunks):
                lo = i * CHUNK
                hi = min(max(NAc, 1), lo + CHUNK)
                t = apool.tile([128, hi - lo], bf16, tag=f"ca{i}")
                if i < 2:
                    nc.gpsimd.dma_start(t[:], slabA_d.ap()[:, lo:hi])
                else:
                    qn = min(qload, key=qload.get)
                    qload[qn] += (hi - lo) * 256
                    qeng[qn].dma_start(t[:], slabA_d.ap()[:, lo:hi])
                chA.append(t)
            b0 = bpool.tile([128, max(LB, 1)], bf16, tag="b0")
            b1 = bpool.tile([128, max(LB, 1)], bf16, tag="b1")
            for half, bt in ((0, b0), (1, b1)):
                qn = min(qload, key=qload.get)
                qload[qn] += max(LB, 1) * 256
                qeng[qn].dma_start(bt[:], slabB_d.ap()[128 * half:128 * (half + 1), :])

            copy_i = 0
            for b in range(nb):
                staging = stpool.tile([128, STAGE_W], bf16)
                for g in range(4):
                    Tbase = b * 16 + 4 * g
                    if PSUM_W * Tbase >= Ntot:
                        continue  # tail: staging garbage, never gathered
                    ps = pspool.tile([128, PSUM_W], f32)
                    for a in range(4):
                        T = Tbase + a
                        lo_g = PSUM_W * T
                        if lo_g >= Ntot:
                            continue
                        for s in segs[T]:
                            if s[0] == 0:
                                _, o, w, pid, ci = s
                                off_in_chunk = lo_g + o - ci * CHUNK
                                nc.tensor.matmul(
                                    ps[32 * a:32 * a + 32, o:o + w],
                                    wdict_t[:, 32 * pid:32 * pid + 32],
                                    chA[ci][:, off_in_chunk:off_in_chunk + w],
                                    start=True, stop=True,
                                    tile_position=(0, 32 * a),
                                )
                            else:
                                _, o, w, boff = s
                                nc.tensor.matmul(ps[32 * a:32 * a + 32, o:o + w],
                                                 wf0, b0[:, boff:boff + w],
                                                 start=True, stop=False,
                                                 tile_position=(0, 32 * a))
                                nc.tensor.matmul(ps[32 * a:32 * a + 32, o:o + w],
                                                 wf1, b1[:, boff:boff + w],
                                                 start=False, stop=True,
                                                 tile_position=(0, 32 * a))
                    dst = staging[:, 512 * g:512 * g + 512]
                    if copy_i % 2 == 0:
                        nc.vector.tensor_copy(dst, ps[:])
                    else:
                        nc.scalar.copy(dst, ps[:])
                    copy_i += 1
                nc.gpsimd.dma_start(out_d.ap()[b], staging[:])

    nc.compile()
    return nc


def _get_program(desc):
    key = (desc["NAc"], desc["LB"], desc["Ntot"], desc["nb"],
           tuple(tuple(map(tuple, s)) for s in desc["segs"]))
    if key not in _prog_cache:
        _prog_cache[key] = _build_program(desc)
    return _prog_cache[key]


def _pack(x, W, k_idx, out_idx, num_out):
    """Host-side packing. Returns (in_maps, desc, vox_core, vox_col)."""
    import ml_dtypes
    bf = ml_dtypes.bfloat16
    n = x.shape[0]

    # occupied-slot masks per output voxel
    masks = np.zeros(num_out, np.uint8)
    np.bitwise_or.at(masks, out_idx, (np.uint8(1) << k_idx.astype(np.uint8)))
    m = POP[masks]

    # class A (m<=4): canonical 4-bit pattern = mask padded with lowest unset bits
    isA = m <= 4
    Aids = np.where(isA)[0]
    Bids = np.where(~isA)[0]
    cntA, cntB = len(Aids), len(Bids)

    pat = masks.astype(np.int64).copy()
    need = 4 - m
    for bbit in range(8):
        unset = (pat >> bbit) & 1 == 0
        add = unset & (need > 0) & isA
        pat += add.astype(np.int64) << bbit
        need = need - add
    pid_vox = PAT_ID[pat]  # valid for A voxels

    # deal A voxels round-robin within each pattern run
    pidA = pid_vox[Aids]
    orderA = np.argsort(pidA, kind="stable")
    sorted_ids = Aids[orderA]
    sorted_pid = pidA[orderA]
    cnts = np.bincount(sorted_pid, minlength=70)
    L = -(-cnts // N_CORES)                      # per-core run length
    offs = np.concatenate([[0], np.cumsum(L)[:-1]])
    starts = np.concatenate([[0], np.cumsum(cnts)[:-1]])
    r = np.arange(cntA) - starts[sorted_pid]
    vox_core = np.empty(num_out, np.int64)
    vox_col = np.empty(num_out, np.int64)
    vox_core[sorted_ids] = r % N_CORES
    vox_col[sorted_ids] = offs[sorted_pid] + r // N_CORES
    NAc = int(L.sum())

    rB = np.arange(cntB)
    vox_core[Bids] = rB % N_CORES
    vox_col[Bids] = NAc + rB // N_CORES
    LB = int(-(-cntB // N_CORES))

    Ntot = NAc + LB
    nb = max(1, -(-Ntot // (16 * PSUM_W)))

    # runs -> per-psum-tile segments
    nT = nb * 16
    segs = [[] for _ in range(nT)]
    for pid in range(70):
        if cnts[pid] == 0:
            continue
        off, run_len = int(offs[pid]), int(L[pid])
        lo = off
        while lo < off + run_len:
            T = lo // PSUM_W
            hi = min(off + run_len, (T + 1) * PSUM_W)
            segs[T].append((0, lo - PSUM_W * T, hi - lo, pid, lo // CHUNK))
            lo = hi
    lo = NAc
    while lo < Ntot:
        T = lo // PSUM_W
        hi = min(Ntot, (T + 1) * PSUM_W)
        segs[T].append((1, lo - PSUM_W * T, hi - lo, lo - NAc))
        lo = hi

    # fill slabs
    pairs = out_idx.astype(np.int64) * 8 + k_idx
    unique_pairs = np.unique(pairs).size == n

    ptA = isA[out_idx]
    xa = x[ptA]
    va = out_idx[ptA]
    ka = k_idx[ptA].astype(np.int64)
    rank = POP[pat[va] & ((np.int64(1) << ka) - 1)]
    slabA = np.zeros((N_CORES, max(NAc, 1), 4, 32), np.float32)
    if unique_pairs:
        slabA[vox_core[va], vox_col[va], rank] = xa
    else:
        np.add.at(slabA, (vox_core[va], vox_col[va], rank), xa)

    xb = x[~ptA]
    vb = out_idx[~ptA]
    kb = k_idx[~ptA].astype(np.int64)
    slabB = np.zeros((N_CORES, max(LB, 1), 8, 32), np.float32)
    if unique_pairs:
        slabB[vox_core[vb], vox_col[vb] - NAc, kb] = xb
    else:
        np.add.at(slabB, (vox_core[vb], vox_col[vb] - NAc, kb), xb)

    # weight dictionary: wdict[32r+c, 32p+d] = W[k_r(p), c, d]
    wd = np.zeros((4, 32, 70, 32), np.float32)
    for p, byte in enumerate(PAT4):
        ks = [k for k in range(8) if (byte >> k) & 1]
        for rr, k in enumerate(ks):
            wd[rr, :, p, :] = W[k]
    wdict = np.ascontiguousarray(wd.reshape(128, 70 * 32)).astype(bf)
    wflat = W.reshape(256, 32).astype(bf)

    in_maps = []
    for c in range(N_CORES):
        in_maps.append({
            "slabA": np.ascontiguousarray(
                slabA[c].reshape(max(NAc, 1), 128).T).astype(bf),
            "slabB": np.ascontiguousarray(
                slabB[c].reshape(max(LB, 1), 256).T).astype(bf),
            "wdict": wdict,
            "wflat": wflat,
        })
    desc = {"NAc": NAc, "LB": LB, "Ntot": Ntot, "nb": nb, "segs": segs}
    return in_maps, desc, vox_core, vox_col


def _decode(results, desc, vox_core, vox_col):
    nb = desc["nb"]
    NT = nb * 16 * PSUM_W
    outs = []
    for rres in results:
        st = np.asarray(rres["out_st"], dtype=np.float32)  # [nb, 128, 2048]
        arr = st.reshape(nb, 4, 32, 4, 512)                # [b, a, d, g, t]
        outT = arr.transpose(2, 0, 3, 1, 4).reshape(32, NT)
        outs.append(outT)
    full = np.stack(outs)                                  # [cores, 32, NT]
    return np.ascontiguousarray(full[vox_core, :, vox_col])


def run(x, W, k_idx, out_idx, num_out, trace=False, dt_name=None):
    from concourse.bass_utils import run_bass_kernel_spmd

    x = np.asarray(x, dtype=np.float32)
    W = np.asarray(W, dtype=np.float32)
    k_idx = np.asarray(k_idx, dtype=np.int32)
    out_idx = np.asarray(out_idx, dtype=np.int32)
    num_out = int(num_out)

    in_maps, desc, vox_core, vox_col = _pack(x, W, k_idx, out_idx, num_out)
    nc = _get_program(desc)
    res = run_bass_kernel_spmd(nc, in_maps, list(range(N_CORES)), trace=trace)
    out = _decode(res.results, desc, vox_core, vox_col)
    return out, res


def kernel(x, W, k_idx, out_idx, num_out):
    out, _ = run(x, W, k_idx, out_idx, num_out, trace=False)
    return out

